# revision 34
# baseline (speedup 1.0000x reference)
"""Causal self-attention (B=2, N=2048, D=1024, H=16) on 8 trn2 NeuronCores.

Sharding: data-parallel over batch (2) x tensor-parallel over heads (4 head
groups of 4 heads) = 8 cores. Each core computes QKV projection for its 4
heads, causal attention, and its partial of the output projection (contraction
over its heads' dims). The host sums the 4 partials per batch element and adds
the constant term (out bias + v-bias routed through W_out, exact because
softmax rows sum to 1).

v3 notes (measured on HW, not the cost model):
  - bf16 tiles for x/weights/q/k/v/aoT: full PE rate at any free dim, fast
    weight loads, halved DMA bytes. fp16 is NOT used: ACT/DVE writes
    converting fp32->fp16 run ~19% slower; bf16 is a truncation and is fast.
  - p (exp output) stays fp32: the ACT exp writing bf16 costs +14%/element on
    the critical path; fp32r matmuls at free dim >= 256 are full rate anyway.
    Only the 128-wide diagonal p@v pays the 4x fp32r penalty (~2.5us total).
  - x is loaded as 8 persistent [128, 2048] whole-row tiles up front: DMA ring
    time here is descriptor-bound (128 per tile, independent of row bytes), so
    whole rows move 4x the data for the same ring time as one chunk window.
  - k-projection bias dropped (softmax-invariant); only q gets its bias.
  - Per chunk the diagonal blocks' scores+exp+mask run FIRST, off-diagonal
    p@v stream follows, diagonal p@v accumulate last: the gpsimd causal mask
    and the exp latency never gate the PE at block boundaries.
  - Last chunk: yproj k=0 half runs as fillers during pr=1 (against pr=0's
    already-normalized aoT), parked in SBUF; after pr=1's inline norm only the
    k=1 matmuls + DVE adds + writeback remain.
"""

import sys
from collections import deque

if '/opt/trn_rl_repo' not in sys.path:
    sys.path.insert(0, '/opt/trn_rl_repo')

import numpy as np
from ml_dtypes import bfloat16 as np_bf16

import concourse.bacc as bacc
import concourse.mybir as mybir
import concourse.tile as tile
from concourse.bass_utils import run_bass_kernel_spmd

F32 = mybir.dt.float32
F32R = mybir.dt.float32r
BF16 = mybir.dt.bfloat16
EXP = mybir.ActivationFunctionType.Exp
MULT = mybir.AluOpType.mult
ADD = mybir.AluOpType.add
IS_GE = mybir.AluOpType.is_ge

B, N, D, H = 2, 2048, 1024, 16
HD = D // H          # 64
HPC = 4              # heads per core
NCORES = 8
NT = N // 512        # 4 token chunks of 512
NJ = N // 128        # 16 key tiles of 128
SCALE = HD ** (-0.5)


def _emit(ctx, nc, tc, dram):
    xT, wqk, wv, wo, bq, y = (
        dram['xT'], dram['wqk'], dram['wv'], dram['wo'], dram['bq'],
        dram['y'])

    cp = ctx.enter_context(tc.tile_pool(name="const", bufs=1))
    pp = ctx.enter_context(tc.tile_pool(name="pexp", bufs=4))
    pd = ctx.enter_context(tc.tile_pool(name="pdiag", bufs=5))
    sm = ctx.enter_context(tc.tile_pool(name="small", bufs=4))
    psc = ctx.enter_context(tc.tile_pool(name="psc", bufs=2, space="PSUM"))
    pao = ctx.enter_context(tc.tile_pool(name="pao", bufs=2, space="PSUM"))
    pms = ctx.enter_context(tc.tile_pool(name="pms", bufs=2, space="PSUM"))

    # ---- persistent tiles -------------------------------------------------
    xp = ctx.enter_context(tc.tile_pool(name="xwin", bufs=20))
    wqk_t = [cp.tile([128, 512], BF16, tag=f"wqk{f}", name=f"wqk{f}")
             for f in range(8)]
    wv_t = [cp.tile([128, 256], BF16, tag=f"wv{f}", name=f"wv{f}")
            for f in range(8)]
    wo_t = [cp.tile([128, 1024], BF16, tag=f"wo{k}", name=f"wo{k}")
            for k in range(2)]
    bq_t = [cp.tile([128, 1], F32, tag=f"bq{r}", name=f"bq{r}")
            for r in range(2)]
    # q,k transposed: 4 tiles [128 dims, 2048 tokens]; rt 0,1 = q; rt 2,3 = k
    qkT = [cp.tile([128, N], BF16, tag=f"qkT{r}", name=f"qkT{r}")
           for r in range(4)]
    # v natural per j-tile with ones columns: [v_h0|1|v_h1|1|v_h2|1|v_h3|1]
    # (f32r, matching p: walrus rejects f32r x bf16 matmul input mixes)
    v_t = [cp.tile([128, 65 * HPC], F32R, tag=f"v{j}", name=f"v{j}")
           for j in range(NJ)]
    # normalized attention output, transposed [head dims, tokens]
    aoT = [cp.tile([128, N], BF16, tag=f"aoT{k}", name=f"aoT{k}")
           for k in range(2)]

    # Startup loads. The first matmul chain needs wqk_t[0] + the first x
    # window tile; x window tiles alternate the two HWDGE rings (sync/scalar)
    # since a single ring moves ~48GB/s. Weights ride the later-starting
    # gpsimd SWDGE ring (wqk first -- the qk chains need it before wv/wo).
    nc.scalar.dma_start(out=wqk_t[0][:], in_=wqk[0:128, :])
    for r in range(2):
        nc.scalar.dma_start(out=bq_t[r][:],
                            in_=bq[r * 128:(r + 1) * 128, :])
    for f in range(1, 8):
        nc.gpsimd.dma_start(out=wqk_t[f][:],
                            in_=wqk[f * 128:(f + 1) * 128, :])
    for f in range(8):
        nc.gpsimd.dma_start(out=wv_t[f][:], in_=wv[f * 128:(f + 1) * 128, :])
    for k in range(2):
        nc.gpsimd.dma_start(out=wo_t[k][:], in_=wo[k * 128:(k + 1) * 128, :])

    def start_x_window(c):
        xw = [xp.tile([128, 512], BF16, tag="xw", name=f"xw{c}_{f}")
              for f in range(8)]
        for f in range(8):
            eng = nc.sync if (f % 2 == 0) else nc.scalar
            eng.dma_start(
                out=xw[f][:],
                in_=xT[f * 128:(f + 1) * 128, c * 512:(c + 1) * 512])
        return xw

    # ---- task generators (each yielded thunk emits ~one PE instruction) ---
    def qkv_tasks(c, xw):
        c0 = c * 512
        # q,k transposed r-tiles
        for rt in range(4):
            st = {}
            def _mk(rt, f, st):
                def _t():
                    if f == 0:
                        st['ps'] = pms.tile([128, 512], F32, tag="ms",
                                            name=f"qk{c}_{rt}")
                    nc.tensor.matmul(st['ps'][:],
                                     wqk_t[f][:, rt * 128:(rt + 1) * 128],
                                     xw[f][:], start=(f == 0), stop=(f == 7))
                    if f == 7:
                        if rt < 2:
                            nc.vector.tensor_scalar_add(
                                qkT[rt][:, c0:c0 + 512],
                                st['ps'][:], bq_t[rt][:])
                        else:
                            nc.vector.tensor_copy(
                                out=qkT[rt][:, c0:c0 + 512],
                                in_=st['ps'][:])
                return _t
            for f in range(8):
                yield _mk(rt, f, st)
        # v natural t-tiles
        for tt in range(4):
            jt = 4 * c + tt
            st = {}
            def _mk(tt, jt, f, st):
                def _t():
                    if f == 0:
                        st['ps'] = pms.tile([128, 256], F32, tag="ms",
                                            name=f"v{c}_{tt}")
                    nc.tensor.matmul(st['ps'][:],
                                     xw[f][:, tt * 128:(tt + 1) * 128],
                                     wv_t[f][:], start=(f == 0), stop=(f == 7))
                    if f == 7:
                        ps = st['ps']
                        v3 = v_t[jt][:].rearrange("p (g e) -> p g e", e=65)
                        nc.vector.tensor_scalar(
                            out=v3[:, :, 64:65],
                            in0=ps[:, 0:4].rearrange("p (g e) -> p g e", e=1),
                            scalar1=0.0, scalar2=1.0, op0=MULT, op1=ADD)
                        nc.vector.tensor_copy(
                            out=v3[:, :, 0:64],
                            in_=ps[:].rearrange("p (g e) -> p g e", e=64))
                return _t
            for f in range(8):
                yield _mk(tt, jt, f, st)

    COPYF = mybir.ActivationFunctionType.Copy

    def yproj_tasks(c, copy_scalar=False):
        for tt in range(4):
            t0 = c * 512 + tt * 128
            st = {}
            for ec in range(2):
                def _mk(t0, ec, k, st):
                    def _t():
                        if ec == 0 and k == 0:
                            st['y'] = sm.tile([128, 1024], F32, tag="y",
                                              name=f"y{t0}", bufs=3)
                        if k == 0:
                            st['ps'] = pms.tile([128, 512], F32, tag="ms",
                                                name=f"yp{t0}_{ec}")
                        nc.tensor.matmul(
                            st['ps'][:], aoT[k][:, t0:t0 + 128],
                            wo_t[k][:, ec * 512:(ec + 1) * 512],
                            start=(k == 0), stop=(k == 1))
                        if k == 1:
                            dst = st['y'][:, ec * 512:(ec + 1) * 512]
                            if copy_scalar and (t0 // 128 + ec) % 2 == 0:
                                # end game: alternate evacuation between the
                                # (exp-idle) ScalarE and DVE so the 2-buffer
                                # PSUM ring is never copy-bound.
                                nc.scalar.activation(dst, st['ps'][:], COPYF)
                            else:
                                nc.vector.tensor_copy(out=dst,
                                                      in_=st['ps'][:])
                            nc.sync.dma_start(
                                out=y[t0:t0 + 128,
                                      ec * 512:(ec + 1) * 512],
                                in_=dst)
                    return _t
                for k in range(2):
                    yield _mk(t0, ec, k, st)

    def emit_pv(pr, jt, p_entry, ao_e, ao_o, first, last):
        p, i0 = p_entry
        for g, ao in ((2 * pr, ao_e), (2 * pr + 1, ao_o)):
            nc.tensor.matmul(
                ao[:, i0:512], v_t[jt][:, g * 65:g * 65 + 65],
                p[:, (g % 2) * 512 + i0:(g % 2) * 512 + 512],
                start=first, stop=last)

    def emit_score_exp(c, pr, jt, qt, kt, diag):
        """Score matmul pair + exp (+ causal mask for diagonal tiles).
        Returns the (p_tile, i0) entry for the later p@v."""
        d = jt - 4 * c
        i0 = 128 * d if d >= 1 else 0
        w = 512 - i0
        sc = psc.tile([128, 1024], F32, tag="sc", name=f"sc{c}_{pr}_{jt}")
        nc.tensor.matmul(sc[:, i0:512],
                         kt[0:64, jt * 128:(jt + 1) * 128],
                         qt[0:64, c * 512 + i0:(c + 1) * 512],
                         start=True, stop=True)
        nc.tensor.matmul(sc[:, 512 + i0:1024],
                         kt[64:128, jt * 128:(jt + 1) * 128],
                         qt[64:128, c * 512 + i0:(c + 1) * 512],
                         start=True, stop=True)
        pool = pd if diag else pp
        p = pool.tile([128, 1024], F32R, tag="p", name=f"p{c}_{pr}_{jt}")
        p3 = p[:].rearrange("p (h i) -> p h i", i=512)[:, :, i0:512]
        sc3 = sc[:].rearrange("p (h i) -> p h i", i=512)[:, :, i0:512]
        nc.scalar.activation(p3, sc3, EXP, scale=SCALE)
        if d >= 0:
            nc.gpsimd.affine_select(
                out=p3, in_=p3, compare_op=IS_GE, fill=0.0,
                base=0, channel_multiplier=-1,
                pattern=[[0, 2], [1, w]])
        return (p, i0)

    # ---- main schedule ----------------------------------------------------
    rscratch_t = nc.dram_tensor("rscratch", [16, 1, 512], F32)
    rscratch = [rscratch_t.ap()[i] for i in range(16)]
    # PE warm-up: the HAM clock gate needs ~3.4us of sustained matmul
    # activity to lift the PE clock from 1.2 to 2.4 GHz. These dummy matmuls
    # have no DMA dependencies, so they run while the first weight/x loads
    # are still in flight and the real chains start at full clock. Inputs
    # are uninitialized SBUF, outputs land in a recycled PSUM tile.
    warm = cp.tile([128, 512], BF16, tag="warm", name="warm")
    nc.vector.memset(warm[:], 0.0)
    wps = pms.tile([128, 512], F32, tag="ms", name="warmps")
    for _ in range(16):
        nc.tensor.matmul(wps[:], warm[:, 0:128], warm[:],
                         start=True, stop=True)

    fill = deque()
    xw0 = start_x_window(0)
    # chunk 0 is gated by the x window DMAs: run chains in f-interleaved
    # pairs (2 PSUM accumulators) so each arriving x tile feeds two matmuls
    tasks0 = list(qkv_tasks(0, xw0))
    for base in (0, 16, 32, 48):
        for f in range(8):
            tasks0[base + f]()
            tasks0[base + 8 + f]()

    for c in range(NT):
        if c + 1 < NT:
            xw_next = start_x_window(c + 1)
            fill.extend(qkv_tasks(c + 1, xw_next))
        # out-projections run late on purpose: the last chunk has no qkv
        # fillers left and its blocks are otherwise exp-bound on ScalarE.
        if c == 2:
            fill.extend(yproj_tasks(0))
        elif c == 3:
            fill.extend(yproj_tasks(1))
            fill.extend(yproj_tasks(2))

        njt = 4 * c + 4
        nblocks = 2 * njt
        blk = 0
        # last chunk: hold 16 filler matmuls back so the inline norm chain's
        # latency before the final out-projection is covered with PE work
        reserve = 12 if c < NT - 1 else 16

        def drain_fillers(blocks_left, reserve=reserve):
            avail = max(0, len(fill) - reserve)
            want = -(-avail // max(blocks_left, 1))  # ceil
            for _ in range(min(want, avail)):
                fill.popleft()()

        for pr in range(2):
            qt, kt = qkT[pr], qkT[2 + pr]
            ao_e = pao.tile([65, 512], F32, tag="ao", name=f"aoe{c}_{pr}")
            ao_o = pao.tile([65, 512], F32, tag="ao", name=f"aoo{c}_{pr}")

            # ascending j-tiles, one sc allocation per block: the 2-deep sc
            # PSUM pool plus the FIFO tensor queue means any deeper lookahead
            # of exp work stalls the PE at the queue head (measured, twice).
            plist = []
            for jt in range(njt):
                plist.append(emit_score_exp(c, pr, jt, qt, kt,
                                            jt >= 4 * c))
                if jt >= 1:
                    emit_pv(pr, jt - 1, plist[jt - 1], ao_e, ao_o,
                            first=(jt - 1 == 0), last=False)
                blk += 1
                drain_fillers(nblocks - blk)
            emit_pv(pr, njt - 1, plist[njt - 1], ao_e, ao_o,
                    first=False, last=True)

            # normalization
            for g, ao in ((2 * pr, ao_e), (2 * pr + 1, ao_o)):
                if c == NT - 1 and pr == 1:
                    # end-game inline norm: split the serial chain across
                    # ScalarE (idle once exp is done) + DVE + gpsimd so the
                    # final yproj starts as early as possible.
                    s_row = sm.tile([1, 512], F32, tag="srow",
                                    name=f"srow{c}_{g}", bufs=2)
                    t = sm.tile([65, 512], F32, tag="aosb",
                                name=f"aosb{c}_{g}")
                    if g % 2 == 0:
                        nc.scalar.activation(s_row[:], ao[64:65, :], COPYF)
                        nc.scalar.activation(t[:], ao[:], COPYF)
                    else:
                        nc.vector.tensor_copy(out=s_row[:], in_=ao[64:65, :])
                        nc.scalar.activation(t[:], ao[:], COPYF)
                    r = sm.tile([1, 512], F32, tag="r", name=f"r{c}_{g}",
                                bufs=2)
                    nc.vector.reciprocal_approx_fast(out=r[:], in_=s_row[:])
                    rb = sm.tile([64, 512], F32, tag="rb", name=f"rb{c}_{g}",
                                 bufs=2)
                    nc.gpsimd.partition_broadcast(rb[:], r[:])
                    nc.vector.tensor_mul(
                        aoT[pr][(g % 2) * 64:(g % 2) * 64 + 64,
                                c * 512:(c + 1) * 512],
                        t[0:64, :], rb[:])
                    continue
                t = sm.tile([65, 512], F32, tag="aosb", name=f"aosb{c}_{g}")
                nc.vector.tensor_copy(out=t[:], in_=ao[:])

                def _norm_tail(c=c, pr=pr, g=g, t=t):
                    r = sm.tile([1, 512], F32, tag="r", name=f"r{c}_{g}", bufs=2)
                    # custom-DVE ops don't handle nonzero partition
                    # offsets; stage the sums row at partition 0 first.
                    s_row = sm.tile([1, 512], F32, tag="srow",
                                    name=f"srow{c}_{g}", bufs=2)
                    nc.vector.tensor_copy(out=s_row[:], in_=t[64:65, :])
                    nc.vector.reciprocal_approx_fast(out=r[:], in_=s_row[:])
                    # broadcast R across 64 partitions via a DRAM bounce
                    # (keeps gpsimd free for the causal-mask selects; an
                    # SBUF-source broadcast AP is not expressible).
                    rb = sm.tile([64, 512], F32, tag="rb", name=f"rb{c}_{g}", bufs=2)
                    if c == NT - 1:
                        # end game: gpsimd is idle and lower-latency here
                        nc.gpsimd.partition_broadcast(rb[:], r[:])
                    else:
                        rd = rscratch[4 * c + g]
                        nc.sync.dma_start(out=rd, in_=r[:])
                        nc.sync.dma_start(out=rb[:],
                                          in_=rd.to_broadcast([64, 512]))
                    nc.vector.tensor_mul(
                        aoT[pr][(g % 2) * 64:(g % 2) * 64 + 64,
                                c * 512:(c + 1) * 512],
                        t[0:64, :], rb[:])
                if c == NT - 1:
                    # jump the queue: aoT[0] must be normalized well before
                    # the final out-projection reads it.
                    fill.appendleft(_norm_tail)
                else:
                    fill.append(_norm_tail)

        if c == NT - 1:
            # drain whatever fillers remain, then the final out-projection
            # (PSUM evacuation on the now-idle ScalarE)
            while fill:
                fill.popleft()()
            for t in yproj_tasks(NT - 1, copy_scalar=True):
                t()

    while fill:
        fill.popleft()()


_CACHE = {}


def _build():
    if 'nc' in _CACHE:
        return _CACHE['nc']
    nc = bacc.Bacc("TRN2", target_bir_lowering=False, debug=False)
    dram = {
        'xT': nc.dram_tensor("xT", [D, N], BF16, kind="ExternalInput").ap(),
        'wqk': nc.dram_tensor("wqk", [D, 512], BF16,
                              kind="ExternalInput").ap(),
        'wv': nc.dram_tensor("wv", [D, 256], BF16, kind="ExternalInput").ap(),
        'wo': nc.dram_tensor("wo", [256, D], BF16, kind="ExternalInput").ap(),
        'bq': nc.dram_tensor("bq", [256, 1], F32, kind="ExternalInput").ap(),
        'y': nc.dram_tensor("y", [N, D], F32, kind="ExternalOutput").ap(),
    }
    from contextlib import ExitStack
    with tile.TileContext(nc) as tc, ExitStack() as ctx:
        _emit(ctx, nc, tc, dram)
    nc.compile()
    _CACHE['nc'] = nc
    return nc


def _prep_core_inputs(x, W_qkv, b_qkv, W_out, core):
    b = core // 4
    h0 = HPC * (core % 4)
    r0 = HD * h0
    q_rows = W_qkv[r0:r0 + 256]
    k_rows = W_qkv[D + r0:D + r0 + 256]
    v_rows = W_qkv[2 * D + r0:2 * D + r0 + 256]
    return {
        'xT': np.ascontiguousarray(x[b].T).astype(np_bf16),
        'wqk': np.ascontiguousarray(
            np.concatenate([q_rows, k_rows], 0).T).astype(np_bf16),
        'wv': np.ascontiguousarray(v_rows.T).astype(np_bf16),
        'wo': np.ascontiguousarray(W_out[:, r0:r0 + 256].T).astype(np_bf16),
        'bq': b_qkv[r0:r0 + 256].reshape(256, 1).astype(np.float32),
    }


def kernel(x, W_qkv, b_qkv, W_out, b_out, _trace=False, _tmpdir=None):
    x = np.asarray(x, dtype=np.float32)
    W_qkv = np.asarray(W_qkv, dtype=np.float32)
    b_qkv = np.asarray(b_qkv, dtype=np.float32)
    W_out = np.asarray(W_out, dtype=np.float32)
    b_out = np.asarray(b_out, dtype=np.float32)

    in_maps = [_prep_core_inputs(x, W_qkv, b_qkv, W_out, c)
               for c in range(NCORES)]
    nc = _build()
    res = run_bass_kernel_spmd(nc, in_maps, list(range(NCORES)),
                               trace=_trace, tmpdir=_tmpdir)

    # v-bias contribution (softmax rows sum to 1) + output bias, as one
    # constant vector added on the host.
    bv = b_qkv[2 * D:3 * D]
    const = (b_out.astype(np.float64)
             + W_out.astype(np.float64) @ bv.astype(np.float64))
    out = np.empty((B, N, D), dtype=np.float32)
    for b in range(B):
        acc = np.zeros((N, D), dtype=np.float64)
        for g in range(4):
            acc += res.results[4 * b + g]['y'].astype(np.float64)
        out[b] = (acc + const).astype(np.float32)
    if _trace:
        kernel.last_exec_time_ns = res.exec_time_ns
        kernel.last_trace = (res.instructions_and_trace[1]
                             if res.instructions_and_trace else None)
    return out
